# revision 5
# baseline (speedup 1.0000x reference)
"""Trainium2 Bass kernel v8 for sliding-window GQA attention.

Structure (per 512-token block tb):
  A(0,tb) logits+tanh -> proj(tb+1) (incl. PE transposes of x) -> B(0,tb)
so the PE never waits on the Act engine. After tb3: AllToAll for q-head 0
overlapped by a software-pipelined lh=1-only attention sweep, then the
second AllToAll, then the output projection with 3-deep wo prefetch that
reuses the freed w/x SBUF space.

- all matmul operands bf16 (rel err ~5e-3 vs 2e-2 budget)
- x in 4 plain DMAs per block; PE transposes via identity (no XBAR)
- softmax denominator: ones-matmul -> DVE reciprocal -> gpsimd
  partition_broadcast (Act only runs tanh+exp; no table thrash)
- output written bf16, host upcasts

Sharding: core i owns q-heads {2i, 2i+1}, kv-head i; output rows
[256*i, 256*(i+1)) after two by-head AllToAlls.
"""
import sys

if '/opt/trn_rl_repo' not in sys.path:
    sys.path.insert(0, '/opt/trn_rl_repo')

import numpy as np
import ml_dtypes

import concourse.bass as bass
import concourse.mybir as mybir
import concourse.tile as tile
from concourse import bacc
from concourse.bass_utils import run_bass_kernel_spmd

f32 = mybir.dt.float32
bf16 = mybir.dt.bfloat16
i32 = mybir.dt.int32
AF = mybir.ActivationFunctionType
Alu = mybir.AluOpType

N_CORES = 8
T, D, HD = 2048, 3584, 256
DC = D // 128
TB = 512
NTB = T // TB
SOFT_CAP = 50.0
QUERY_SCALAR = 0.0625
WINDOW = 1024
MASK_VAL = -1.0e6
TANH_SCALE = QUERY_SCALAR / SOFT_CAP

CAUSAL_DD = (0, 128, 256, 384)
WINDOW_DD = (-1024, -896, -768, -640)


def _live_chunks(tb):
    t0 = tb * TB
    smin = max(0, t0 - (WINDOW - 1))
    smax = t0 + TB - 1
    return list(range(smin // 128, smax // 128 + 1))


def _build_module():
    nc = bacc.Bacc("TRN2", target_bir_lowering=False, debug=False,
                   num_devices=N_CORES)

    x_in = nc.declare_dram_parameter("x", [T, D], bf16, isOutput=False)
    wqkv_in = nc.declare_dram_parameter("wqkv", [D, 1024], bf16, isOutput=False)
    wo_in = nc.declare_dram_parameter("wo", [4096, D], bf16, isOutput=False)
    # [sin T | cos T | ones 1 | identity 128]
    sincos_in = nc.declare_dram_parameter("sincos", [128, 2 * T + 129], bf16,
                                          isOutput=False)
    out_ext = nc.declare_dram_parameter("out", [T // N_CORES, D], bf16,
                                        isOutput=True)

    cc_in = [nc.dram_tensor(f"cc_in{h}", [8, 256, 256], bf16) for h in range(2)]
    cc_out = [nc.dram_tensor(f"cc_out{h}", [8, 256, 256], bf16)
              for h in range(2)]

    with tile.TileContext(nc) as tc:
        with (
            tc.tile_pool(name="prep", bufs=1) as prep,
            tc.tile_pool(name="pers", bufs=1) as pers,
            tc.tile_pool(name="pjp", bufs=1) as pjp,
            tc.tile_pool(name="encp", bufs=1) as encp,
            tc.tile_pool(name="ps_mm", bufs=3, space="PSUM") as ps_mm,
            tc.tile_pool(name="ps_acc", bufs=2, space="PSUM") as ps_acc,
            tc.tile_pool(name="ps_tr", bufs=2, space="PSUM") as ps_tr,
            tc.tile_pool(name="ps_den", bufs=1, space="PSUM") as ps_den,
        ):
            ones_col_b = prep.tile([128, 1], bf16)
            nc.sync.dma_start(ones_col_b[:], sincos_in[:, 2 * T:2 * T + 1])
            ident_b = prep.tile([128, 128], bf16)
            nc.sync.dma_start(ident_b[:], sincos_in[:, 2 * T + 1:2 * T + 129])
            sin_t = pers.tile([128, T], bf16)
            nc.scalar.dma_start(sin_t[:], sincos_in[:, 0:T])
            cos_t = pers.tile([128, T], bf16)
            nc.scalar.dma_start(cos_t[:], sincos_in[:, T:2 * T])

            qT_sb = [pers.tile([128, T], bf16, tag=f"qT{i}", name=f"qT{i}")
                     for i in range(4)]
            kT_sb = [pers.tile([128, T], bf16, tag=f"kT{i}", name=f"kT{i}")
                     for i in range(2)]
            v_sb = pers.tile([128, 16 * 256], bf16)

            mask_tiles = {}
            for dd in CAUSAL_DD:
                m = pers.tile([128, 512], bf16, tag=f"mc{dd}", name=f"mc{dd}")
                nc.gpsimd.memset(m[:], 0.0)
                nc.gpsimd.affine_select(
                    out=m[:], in_=m[:], compare_op=Alu.is_ge, fill=MASK_VAL,
                    base=-dd, pattern=[[1, 512]], channel_multiplier=-1)
                mask_tiles[dd] = m
            for dd in WINDOW_DD:
                m = pers.tile([128, 512], bf16, tag=f"mw{dd}", name=f"mw{dd}")
                nc.gpsimd.memset(m[:], 0.0)
                nc.gpsimd.affine_select(
                    out=m[:], in_=m[:], compare_op=Alu.is_gt, fill=MASK_VAL,
                    base=dd + WINDOW, pattern=[[-1, 512]], channel_multiplier=1)
                mask_tiles[dd] = m

            def attn_a(lh, tb, slot):
                js = _live_chunks(tb)
                ts0 = tb * TB
                q0 = qT_sb[lh * 2][:, ts0:ts0 + TB]
                q1 = qT_sb[lh * 2 + 1][:, ts0:ts0 + TB]
                pjs = []
                for idx, sj in enumerate(js):
                    ps_l = ps_mm.tile([128, 512], f32, tag="mm")
                    nc.tensor.matmul(ps_l[:],
                                     kT_sb[0][:, sj * 128:(sj + 1) * 128],
                                     q0, start=True, stop=False)
                    nc.tensor.matmul(ps_l[:],
                                     kT_sb[1][:, sj * 128:(sj + 1) * 128],
                                     q1, start=False, stop=True)
                    dd = sj * 128 - ts0
                    if dd in mask_tiles:
                        nc.vector.tensor_tensor(ps_l[:], ps_l[:],
                                                mask_tiles[dd][:], Alu.add)
                    pj = pjp.tile([128, 512], bf16, tag=f"pj{slot}_{idx}",
                                  name=f"pj{slot}_{idx}")
                    nc.scalar.activation(pj[:], ps_l[:], AF.Tanh,
                                         scale=TANH_SCALE)
                    pjs.append(pj)
                return pjs

            def attn_b(lh, tb, pjs):
                js = _live_chunks(tb)
                for pj in pjs:
                    nc.scalar.activation(pj[:], pj[:], AF.Exp, scale=SOFT_CAP)
                e0 = ps_acc.tile([128, 512], f32, tag="acc")
                e1 = ps_acc.tile([128, 512], f32, tag="acc")
                den = ps_den.tile([1, 512], f32, tag="den")
                for idx, sj in enumerate(js):
                    first, last = idx == 0, idx == len(js) - 1
                    nc.tensor.matmul(e0[:], v_sb[:, sj * 256:sj * 256 + 128],
                                     pjs[idx][:], start=first, stop=last)
                    nc.tensor.matmul(e1[:],
                                     v_sb[:, sj * 256 + 128:sj * 256 + 256],
                                     pjs[idx][:], start=first, stop=last)
                    nc.tensor.matmul(den[:], ones_col_b[:], pjs[idx][:],
                                     start=first, stop=last)
                recip = encp.tile([1, 512], f32, tag="recip")
                nc.vector.reciprocal(recip[:], den[:])
                bc_sb = encp.tile([128, 512], f32, tag="bc_sb")
                nc.gpsimd.partition_broadcast(bc_sb[:], recip[:])
                for c, e_ps in enumerate((e0, e1)):
                    e_sb = encp.tile([128, 512], bf16, tag=f"e_sb{c}")
                    nc.vector.tensor_tensor(e_sb[:], e_ps[:], bc_sb[:],
                                            Alu.mult)
                    r0 = c * 128
                    nc.gpsimd.dma_start(cc_in[lh][tb * 2, r0:r0 + 128, :],
                                        e_sb[:, 0:256])
                    nc.gpsimd.dma_start(cc_in[lh][tb * 2 + 1, r0:r0 + 128, :],
                                        e_sb[:, 256:512])

            # ---------- heavy pool: weights, x, transposes, projections -----
            with (
                tc.tile_pool(name="heavy", bufs=1) as heavy,
                tc.tile_pool(name="xinp", bufs=1) as xinp,
                tc.tile_pool(name="xtp", bufs=1) as xtp,
                tc.tile_pool(name="rope", bufs=2) as rope,
            ):
                w7 = []
                for i in range(7):
                    wt = heavy.tile([128, 4096], bf16, tag=f"w{i}",
                                    name=f"w{i}")
                    nc.scalar.dma_start(
                        wt[:].rearrange("p (dc f) -> p dc f", dc=4),
                        wqkv_in[i * 512:(i + 1) * 512, :]
                        .rearrange("(dc p) f -> p dc f", p=128))
                    w7.append(wt)

                def w_sl(d, lo, hi):
                    return w7[d // 4][:, (d % 4) * 1024 + lo:(d % 4) * 1024 + hi]

                xt_cur = {}

                def load_x(tb):
                    r0 = tb * TB
                    tiles = []
                    for tc4 in range(4):
                        t_ = xinp.tile([128, D], bf16, tag=f"x{tc4}",
                                       name=f"x{tc4}_{tb}")
                        nc.sync.dma_start(
                            t_[:], x_in[r0 + tc4 * 128:r0 + (tc4 + 1) * 128, :])
                        tiles.append(t_)
                    xt_cur[tb] = tiles

                xT = [xtp.tile([128, 512], bf16, tag=f"xT{d}", name=f"xT{d}")
                      for d in range(DC)]

                def proj_block(tb):
                    ts0 = tb * TB
                    x_t = xt_cur.pop(tb)
                    # PE transpose x -> xT[d] [128 d, 512 t]
                    for d in range(DC):
                        tp = ps_tr.tile([128, 512], bf16, tag="tr")
                        for tc4 in range(4):
                            nc.tensor.transpose(
                                tp[:, tc4 * 128:(tc4 + 1) * 128],
                                x_t[tc4][:, d * 128:(d + 1) * 128], ident_b[:])
                        if d % 2 == 0:
                            nc.vector.tensor_copy(xT[d][:], tp[:])
                        else:
                            nc.scalar.copy(xT[d][:], tp[:])

                    for vp in range(2):
                        ps_v = ps_mm.tile([128, 512], f32, tag="mm")
                        for tc4 in range(2):
                            tl = (vp * 2 + tc4) * 128
                            for d in range(DC):
                                nc.tensor.matmul(
                                    ps_v[:, tc4 * 256:(tc4 + 1) * 256],
                                    xT[d][:, tl:tl + 128],
                                    w_sl(d, 768, 1024),
                                    start=(d == 0), stop=(d == DC - 1))
                        sj0 = tb * 4 + vp * 2
                        nc.vector.tensor_copy(
                            v_sb[:, sj0 * 256:(sj0 + 2) * 256], ps_v[:])

                    cos_s = cos_t[:, ts0:ts0 + TB]
                    sin_s = sin_t[:, ts0:ts0 + TB]
                    for pi_, w_off in enumerate((0, 256, 512)):
                        pab = []
                        for half in range(2):
                            ps_qk = ps_mm.tile([128, 512], f32, tag="mm")
                            for d in range(DC):
                                nc.tensor.matmul(
                                    ps_qk[:],
                                    w_sl(d, w_off + half * 128,
                                         w_off + half * 128 + 128),
                                    xT[d][:], start=(d == 0),
                                    stop=(d == DC - 1))
                            pab.append(ps_qk)
                        pA, pB = pab
                        t1 = rope.tile([128, 512], f32, tag="t1")
                        t2 = rope.tile([128, 512], f32, tag="t2")
                        nc.vector.tensor_tensor(t1[:], pA[:], cos_s, Alu.mult)
                        nc.vector.tensor_tensor(t2[:], pB[:], sin_s, Alu.mult)
                        if pi_ < 2:
                            dst0 = qT_sb[pi_ * 2][:, ts0:ts0 + TB]
                            dst1 = qT_sb[pi_ * 2 + 1][:, ts0:ts0 + TB]
                        else:
                            dst0 = kT_sb[0][:, ts0:ts0 + TB]
                            dst1 = kT_sb[1][:, ts0:ts0 + TB]
                        nc.vector.tensor_tensor(dst0, t1[:], t2[:],
                                                Alu.subtract)
                        nc.vector.tensor_tensor(t1[:], pB[:], cos_s, Alu.mult)
                        nc.vector.tensor_tensor(t2[:], pA[:], sin_s, Alu.mult)
                        nc.vector.tensor_tensor(dst1, t1[:], t2[:], Alu.add)

                load_x(0)
                proj_block(0)
                load_x(1)
                pjA = attn_a(0, 0, 0)
                proj_block(1)
                attn_b(0, 0, pjA)
                pjA = attn_a(1, 0, 1)
                attn_b(1, 0, pjA)
                load_x(2)
                pjA = attn_a(0, 1, 0)
                proj_block(2)
                attn_b(0, 1, pjA)
                load_x(3)
                pjA = attn_a(0, 2, 0)
                proj_block(3)
            # heavy/x pools closed: space free for wo prefetch

            with (
                tc.tile_pool(name="ge", bufs=1) as ge,
                tc.tile_pool(name="wop", bufs=6) as wop,
                tc.tile_pool(name="o", bufs=2) as op_,
            ):
                attn_b(0, 2, pjA)
                pjA = attn_a(0, 3, 0)
                attn_b(0, 3, pjA)

                nc.gpsimd.collective_compute(
                    "AllToAll", Alu.bypass,
                    replica_groups=[list(range(N_CORES))],
                    ins=[cc_in[0][:]], outs=[cc_out[0][:]])

                wo_bufs = {}

                def load_wo(db, split=False):
                    halves = []
                    for h in range(2):
                        wo_buf = wop.tile([128, 16 * 512], bf16, tag="wo",
                                          name=f"wo_{db}_{h}")
                        for q2 in range(2):
                            qd = h * 2 + q2
                            eng = nc.scalar if (split and h == 1) else nc.sync
                            eng.dma_start(
                                wo_buf[:, q2 * 8 * 512:(q2 + 1) * 8 * 512]
                                .rearrange("p (j c) -> p j c", j=8),
                                wo_in[qd * 1024:(qd + 1) * 1024,
                                      db * 512:(db + 1) * 512]
                                .rearrange("(j p) c -> p j c", p=128))
                        halves.append(wo_buf)
                    wo_bufs[db] = halves

                load_wo(0)
                load_wo(1)

                enc_sb = {}

                def load_enc(lh):
                    e = ge.tile([128, 16 * 256], bf16, tag=f"enc{lh}",
                                name=f"enc{lh}")
                    for ph in range(2):
                        nc.sync.dma_start(
                            e[:, ph * 8 * 256:(ph + 1) * 8 * 256]
                            .rearrange("p (src c) -> p src c", src=8),
                            cc_out[lh][:, ph * 128:(ph + 1) * 128, :]
                            .rearrange("src p c -> p src c"))
                    enc_sb[lh] = e

                # lh=1 sweep (tb1..3), software-pipelined; overlaps AllToAll lh0
                pj_s = {}
                pj_s[1] = attn_a(1, 1, 1)
                pj_s[2] = attn_a(1, 2, 0)
                attn_b(1, 1, pj_s.pop(1))
                load_enc(0)
                load_wo(2)
                pj_s[3] = attn_a(1, 3, 1)
                attn_b(1, 2, pj_s.pop(2))
                attn_b(1, 3, pj_s.pop(3))

                def enc_chunk(j):
                    src, r = divmod(j, 4)
                    lh, ph = divmod(r, 2)
                    off = (ph * 8 + src) * 256
                    return enc_sb[lh][:, off:off + 256]

                # lh0 chunks of db0/db1 accumulate while AllToAll lh1 runs
                LH0 = [j for j in range(32) if (j % 4) // 2 == 0]
                LH1 = [j for j in range(32) if (j % 4) // 2 == 1]
                part = {}
                for db in range(2):
                    for tc2 in range(2):
                        if db == 0:
                            ps_o = ps_acc.tile([128, 512], f32, tag="acc",
                                               name="ps_oa")
                        else:
                            ps_o = ps_mm.tile([128, 512], f32, tag="mm",
                                              name="ps_ob")
                        for n_, j in enumerate(LH0):
                            wb = wo_bufs[db][j // 16]
                            nc.tensor.matmul(
                                ps_o[:],
                                enc_chunk(j)[:, tc2 * 128:(tc2 + 1) * 128],
                                wb[:, (j % 16) * 512:(j % 16 + 1) * 512],
                                start=(n_ == 0), stop=False)
                        part[(db, tc2)] = ps_o

                nc.gpsimd.collective_compute(
                    "AllToAll", Alu.bypass,
                    replica_groups=[list(range(N_CORES))],
                    ins=[cc_in[1][:]], outs=[cc_out[1][:]])
                load_enc(1)

                for db in range(D // 512):
                    if db + 3 < D // 512:
                        load_wo(db + 3, split=True)
                    wo_halves = wo_bufs.pop(db)
                    for tc2 in range(2):
                        if db < 2:
                            ps_o = part.pop((db, tc2))
                            js_rest, st = LH1, False
                        else:
                            ps_o = ps_mm.tile([128, 512], f32, tag="mm")
                            js_rest, st = list(range(32)), True
                        for n_, j in enumerate(js_rest):
                            wb = wo_halves[j // 16]
                            nc.tensor.matmul(
                                ps_o[:],
                                enc_chunk(j)[:, tc2 * 128:(tc2 + 1) * 128],
                                wb[:, (j % 16) * 512:(j % 16 + 1) * 512],
                                start=(st and n_ == 0),
                                stop=(n_ == len(js_rest) - 1))
                        o_sb = op_.tile([128, 512], bf16, tag="o_sb")
                        nc.vector.tensor_copy(o_sb[:], ps_o[:])
                        nc.scalar.dma_start(
                            out_ext[tc2 * 128:(tc2 + 1) * 128,
                                    db * 512:(db + 1) * 512], o_sb[:])

    nc.compile()
    return nc


_CACHE = {}
LAST_RESULTS = None


def _get_module():
    if "nc" not in _CACHE:
        _CACHE["nc"] = _build_module()
    return _CACHE["nc"]


def kernel(x, segment_pos, attn_mask, wq, wkv, wo):
    global LAST_RESULTS
    bf = ml_dtypes.bfloat16
    x = np.asarray(x, dtype=np.float32)
    segment_pos = np.asarray(segment_pos, dtype=np.int32)
    wq = np.asarray(wq, dtype=np.float32)
    wkv = np.asarray(wkv, dtype=np.float32)
    wo = np.asarray(wo, dtype=np.float32)

    nc = _get_module()

    inv_ts = (10000.0 ** (-np.arange(128, dtype=np.float64) / 128.0))
    ang = segment_pos[0].astype(np.float64)[None, :] * inv_ts[:, None]
    sincos = np.zeros((128, 2 * T + 129), dtype=bf)
    sincos[:, 0:T] = np.sin(ang)
    sincos[:, T:2 * T] = np.cos(ang)
    sincos[:, 2 * T] = 1.0
    sincos[:, 2 * T + 1:] = np.eye(128)

    x2d = np.ascontiguousarray(x[0]).astype(bf)
    wo_flat = np.ascontiguousarray(wo.reshape(4096, D)).astype(bf)

    in_maps = []
    for i in range(N_CORES):
        wqkv = np.concatenate([wq[2 * i], wq[2 * i + 1],
                               wkv[0, i], wkv[1, i]], axis=1)
        in_maps.append({
            "x": x2d,
            "wqkv": np.ascontiguousarray(wqkv).astype(bf),
            "wo": wo_flat,
            "sincos": sincos,
        })

    LAST_RESULTS = run_bass_kernel_spmd(nc, in_maps,
                                        core_ids=list(range(N_CORES)))
    out = np.concatenate([LAST_RESULTS.results[i]["out"]
                          for i in range(N_CORES)], axis=0)
    return out[None, :, :].astype(np.float32)
